# revision 5
# baseline (speedup 1.0000x reference)
"""MEX (log-mean-exp) 3x3 pooling kernel for Trainium2, 8-core data-parallel. v10.

Math: out[n,i,h,w] = log( (1/K) * sum_{c,kh,kw} exp(x[n,c,h+kh-1,w+kw-1] + o[i,c,kh,kw]) )
with zero-padded x OOB (contributing exp(0+o) = exp(o)) and K = 32*3*3 = 288.

Factorization (EPS=1, no max-subtraction needed in f32 range):
    out = log( (1/K) * conv3x3( exp(xpad), exp(o) ) )
where exp(xpad) has 1.0 at padding (= exp(0)).

v4 design: HW probes showed DMA bandwidth collapses ~6x when the access
pattern's leading dim is small (s-major [4,32,...] -> ~89 GB/s vs ~500 for
partition-monotonic shapes) -- SDMA engine fan-out follows the leading dim.
So every transfer here leads with a >=32 partition-aligned dim:
  - in: per-strip [32c, rows*128] loads (SP ring), halos via gpsimd SWDGE
  - out: per (strip, h-group) [32i, 16*128] stores (ACT ring), each 16
    contiguous output rows per instance, overlapping later compute
Compute mapping (one image per core, strips s of 32 rows on partition
group s, es[(s,c)=128, 34, 132] bf16 = exp(xpad), slot k <-> row 32s+k-1):
  - conv as 9 accumulating matmuls (kh,kw), contraction c=32, kh/kw as
    free-dim offsets. Round tq=4h+t computes rows 32s+16h+4t..+3 on 4
    concurrent chains (rg=s, cg=(s+2h)%4) via 32x32 PE array tiling,
    4 chains share one PSUM bank at disjoint partition ranges.
  - One Ln (scale=1/288) per round into the h-group's output staging tile.
  - A manual LoadActFuncSet(natural_log_exp_and_others) keeps Exp and Ln
    co-resident: no activation-table reloads.
"""

import numpy as np

import concourse.bacc as bacc
import concourse.tile as tile
import concourse.mybir as mybir
from concourse.bass_utils import run_bass_kernel_spmd

F32 = mybir.dt.float32
BF16 = mybir.dt.bfloat16
AF = mybir.ActivationFunctionType

N, C, H, W = 8, 32, 128, 128
I = 32
K = C * 3 * 3          # 288
S = 4                  # row strips of 32 output rows
SR = 34                # slots per strip (32 rows + 2 halo)
WP = 132               # padded es width (cols 0..129 used, 130/131 slack)
SPLIT = 19             # slot boundary of the two load/exp chunks

def _act_set_with_exp_ln(arch):
    """Index of an activation-function set containing both Exp and Ln, so one
    LoadActFuncSet covers the whole kernel (no per-round table reloads).
    Falls back to None (compiler inserts correct loads) if unavailable."""
    try:
        from concourse.hw_specs import get_activation_tables
        want = {AF.Exp, AF.Ln}
        for idx, funcs in enumerate(get_activation_tables(arch).values()):
            if want <= funcs:
                return idx
    except Exception:
        pass
    return None


def _build(repeats: int = 1):
    nc = bacc.Bacc("TRN2", target_bir_lowering=False, debug=False)
    x = nc.dram_tensor("x", [C, H, W], F32, kind="ExternalInput").ap()
    off = nc.dram_tensor("offsets", [1, I, C, 3, 3], F32, kind="ExternalInput").ap()
    out = nc.dram_tensor("out", [I, H, W], F32, kind="ExternalOutput").ap()

    # out rows h = 32s + 16hh + 4t + r; (t r w) merges to 2048 contiguous
    out_v = out.rearrange("i (s hh t r) w -> s hh i (t r w)", s=S, hh=2, t=4, r=4)

    with tile.TileContext(nc) as tc:
        with (
            tc.tile_pool(name="const", bufs=1) as constp,
            tc.tile_pool(name="xf", bufs=2) as xfp,
            tc.tile_pool(name="es", bufs=2) as esp,
            tc.tile_pool(name="ps", bufs=8, space="PSUM") as psp,
            tc.tile_pool(name="ob", bufs=3) as obp,
        ):
            # keep exp+ln co-resident on the ACT engine for the whole kernel
            act_set = _act_set_with_exp_ln(nc.m.arch)
            if act_set is not None:
                nc.scalar.add_instruction(
                    mybir.InstLoadActFuncSet(
                        name=nc.get_next_instruction_name(),
                        act_func_set_id=act_set,
                        ins=[],
                        outs=[],
                    )
                )

            # weights wb[(s,c), j=(kh,kw), i] = exp(o[i,c,kh,kw]), replicated
            # across the 4 strip partition groups (one per PE row group);
            # loaded via SWDGE to keep the SP HWDGE ring free for x
            wf = constp.tile([128, 9, I], F32)
            wb = constp.tile([128, 9, I], BF16)
            off_r = off[0].rearrange("i c kh kw -> c (kh kw) i")
            for s in range(S):
                nc.gpsimd.dma_start(wf[32 * s : 32 * s + 32], off_r)
            nc.scalar.activation(wb[:], wf[:], AF.Exp)

            for _rep in range(repeats):
                _emit_body(nc, tc, x, out_v, wb, xfp, esp, psp, obp)
    nc.compile()
    return nc


def _emit_body(nc, tc, x, out_v, wb, xfp, esp, psp, obp):
    xp = xfp.tile([128, SR, W], BF16)    # packed rows; gpsimd DMAs cast f32->bf16
    es = esp.tile([128, SR, WP], BF16)   # padded exp(x): cols 1..128 = data

    # out-of-image rows -> 0 (exp gives the pad value 1.0); l/r pad columns
    # of es never pass through exp, memset them to 1.0 directly
    nc.vector.memset(xp[0:32, 0:1, :], 0.0)
    nc.vector.memset(xp[96:128, SR - 1 : SR, :], 0.0)
    nc.vector.memset(es[:, :, 0:1], 1.0)
    nc.vector.memset(es[:, :, 129:132], 1.0)

    # bulk rows: two ~1 MiB loads on the SP ring (slot k <-> row 32s+k-1);
    # halo slots 0/33 (rows 32s-1 / 32s+32) via SWDGE off the ring
    for a, b in ((1, SPLIT), (SPLIT, SR - 1)):   # slots <- rows 32s+a-1..
        for s in range(S):
            nc.gpsimd.dma_start(
                xp[32 * s : 32 * s + 32, a:b, :].rearrange("p r w -> p (r w)"),
                x[:, 32 * s + a - 1 : 32 * s + b - 1, :].rearrange("c h w -> c (h w)"),
            )
    for s in range(1, S):     # slot 0 <- row 32s-1
        nc.gpsimd.dma_start(xp[32 * s : 32 * s + 32, 0, :], x[:, 32 * s - 1, :])
    for s in range(S - 1):    # slot 33 <- row 32s+32
        nc.gpsimd.dma_start(xp[32 * s : 32 * s + 32, SR - 1, :], x[:, 32 * s + 32, :])

    nc.scalar.activation(es[:, 0:SPLIT, 1 : 1 + W], xp[:, 0:SPLIT, :], AF.Exp)
    nc.scalar.activation(es[:, SPLIT:SR, 1 : 1 + W], xp[:, SPLIT:SR, :], AF.Exp)

    for h in range(2):
        ob = obp.tile([128, 4, 4, W], F32, name=f"ob{h}", tag="ob")
        for t in range(4):
            tq = 4 * h + t
            ps = psp.tile([128, 4, W], F32, name=f"ps{h}{t}", tag="ps")
            for j in range(9):
                kh, kw = divmod(j, 3)
                for s in range(S):
                    g = (s + 2 * h) % S
                    nc.tensor.matmul(
                        ps[32 * g : 32 * g + 32, :, :],
                        wb[32 * s : 32 * s + 32, j, :],
                        es[32 * s : 32 * s + 32, 4 * tq + kh : 4 * tq + kh + 4, kw : kw + W],
                        start=(j == 0),
                        stop=(j == 8),
                        # disjoint-partition chains in one bank trip the
                        # partition-unaware zero-region race check
                        skip_group_check=True,
                        tile_position=(32 * s, 32 * g),
                    )
            nc.scalar.activation(ob[:, t], ps[:], AF.Ln, scale=1.0 / K)
        # stores: one [32i, 2048] per (strip, h) -- i-leading AP keeps all
        # SDMA engines fed; partition group g holds strip s = (g+2h)%4
        obm = ob[:].rearrange("p t r w -> p (t r w)")
        for g in range(S):
            s = (g + 2 * h) % S
            nc.scalar.dma_start(out_v[s, h], obm[32 * g : 32 * g + 32])


_NC = None


def _get_nc():
    global _NC
    if _NC is None:
        _NC = _build()
    return _NC


def kernel(x: np.ndarray, offsets: np.ndarray) -> np.ndarray:
    x = np.ascontiguousarray(x, dtype=np.float32)
    offsets = np.ascontiguousarray(offsets, dtype=np.float32)
    nc = _get_nc()
    in_maps = [
        {"x": np.ascontiguousarray(x[i]), "offsets": offsets} for i in range(N)
    ]
    res = run_bass_kernel_spmd(nc, in_maps, list(range(N))).results
    return np.stack([res[i]["out"] for i in range(N)], axis=0)
